# revision 52
# baseline (speedup 1.0000x reference)
"""Linear attention (elu+1 feature map) Bass/Tile kernel for Trainium2.

Full inputs: queries/keys/values [N=8, L/S=8192, H=8, D=64] fp32.
Sharding: data-parallel over N across the 8 NeuronCores (batch i -> core i).
Wire format: fp16 (inputs cast on host, output cast back), halving both
the host<->device transfer and the on-device HBM traffic.

Math per (n, h):
  Q' = elu(Q)+1, K' = elu(K)+1      with elu(x)+1 == relu(x) + min(exp(x), 1)
  KV[d, v] = sum_s K'[s, d] V[s, v]  (the /S, *S in the reference cancel)
  Ksum[d]  = sum_s K'[s, d]
  out[l, v] = (Q'[l, :] @ KV[:, v]) / (Q'[l, :] @ Ksum)
  (EPS=1e-6 is dropped: denominators are >= ~1e3 for any input since
   K', Q' > 0 and Ksum ~ S.)

Kernel structure per core (all DMAs issued from SP except V on ACT):
  Phase 1 (stream K/V, 1024-row chunks): contiguous fp16 DMAs; feature map
    K' = relu(K) + min(exp(K),1) as ACT Exp + DVE tensor_scalar_min (4x) +
    DVE scalar_tensor_tensor (max,add); per 128-row sub-block and head pair
    j, two accumulating matmuls into PSUM kv_ps[j] [128, 129]:
    lhsT=K'_pair x rhs=V_pair (block-diagonal KV) and x rhs=ones (Ksum col).
    NOTE: exactly one start=True per psum tile - start clears has_written
    for the whole bank row, so a second start drops prior accumulation.
  Phase 1.5: cast PSUM -> fp16 block-diag weights w2[j] [128,128] and
    Ksum columns ksc[j] [128,2].
  Phase Q (per 1024 rows, DMA-gated behind the K/V stream via nosync deps):
    DMA-TRANSPOSE Q to [128 f, l] fp16 (16x128 XBAR tiles, ~14ns/tile);
    feature map split as M = min(exp,1) (ACT + GpSimd ts_min) and
    R = relu (DVE ts_max 4x); the "+" of Q' = M + R is folded into phase 2
    as a second accumulating matmul per output (lhsT=M then lhsT=R).
  Phase 2 per 1024-row chunk: 64 tiny denominator matmuls (rhs=ksc) into
    one PSUM bank -> one wide DVE reciprocal -> zr [128, 64] f32; per 256
    rows one p2 PSUM tile [128, 2, 512] (8 matmuls, heads in column order)
    and ONE DVE tensor_tensor multiply against zr broadcast [128,2,8,64]
    -> fp16 out staging; one fp16 out DMA per 1024 rows.

TimelineSim per-core: ~128.8us (baseline kernel: 317.6us).
"""

import functools
import sys

sys.path.insert(0, "/opt/trn_rl_repo")

import numpy as np

import concourse.bass as bass
from concourse.bass import InstructionNameOrderedSet
import concourse.mybir as mybir
import concourse.tile as tile
from concourse import bacc

N, L, S, H, D = 8, 8192, 8192, 8, 64
P = 128
HD = H * D  # 512
FP32 = mybir.dt.float32
FP16 = mybir.dt.float16
AF = mybir.ActivationFunctionType
OP = mybir.AluOpType

UNGATED_QT = 0  # leading Q chunks allowed to interleave with K/V
KC = 1024  # K/V rows per chunk
QC = 1024  # Q rows per transpose chunk
OC = 1024  # out rows per DMA


def build_kernel(L_=L, S_=S, debug=False):
    nc = bacc.Bacc(trn_type="TRN2")
    q_d = nc.dram_tensor("queries", [L_, HD], FP16, kind="ExternalInput")
    k_d = nc.dram_tensor("keys", [S_, HD], FP16, kind="ExternalInput")
    v_d = nc.dram_tensor("values", [S_, HD], FP16, kind="ExternalInput")
    o_d = nc.dram_tensor("out", [L_, HD], FP16, kind="ExternalOutput")
    if debug:
        w2_dbg = nc.dram_tensor("w2_dbg", [P, 512], FP16, kind="ExternalOutput")
        ksc_dbg = nc.dram_tensor("ksc_dbg", [P, 8], FP16, kind="ExternalOutput")
        zr_dbg = nc.dram_tensor("zr_dbg", [P, 64], FP32, kind="ExternalOutput")
        qm_dbg = nc.dram_tensor("qm_dbg", [P, 4 * 1024], FP16, kind="ExternalOutput")
        qr_dbg = nc.dram_tensor("qr_dbg", [P, 4 * 1024], FP16, kind="ExternalOutput")
        qt_dbg = nc.dram_tensor("qt_dbg", [P, 4 * 1024], FP16, kind="ExternalOutput")
        kp_dbg = nc.dram_tensor("kp_dbg", [P, 8, HD], FP16, kind="ExternalOutput")
        ke_dbg = nc.dram_tensor("ke_dbg", [P, 8, HD], FP16, kind="ExternalOutput")

    n_kc = S_ // KC  # 16
    n_qc = L_ // QC  # 8
    lc_per_qc = QC // P  # 8 query sub-chunks of 128 per qchunk
    kc_subs = KC // P  # 4

    with tile.TileContext(nc) as tc:
        with (
            tc.tile_pool(name="consts", bufs=1) as consts,
            tc.tile_pool(name="kdma", bufs=4) as kdma,
            tc.tile_pool(name="vdma", bufs=2) as vdma,
            tc.tile_pool(name="ke", bufs=2) as kep,
            tc.tile_pool(name="kp", bufs=2) as kpp,
            tc.tile_pool(name="qt", bufs=3) as qtp,
            tc.tile_pool(name="qe", bufs=4) as qep,
            tc.tile_pool(name="qp", bufs=4) as qpp,
            tc.tile_pool(name="w2", bufs=1) as w2p,
            tc.tile_pool(name="zr", bufs=3) as zrp,
            tc.tile_pool(name="otile", bufs=3) as outp,
        ):
            ones = consts.tile([P, 1], FP16)
            nc.vector.memset(ones, 1.0)

            qps = []
            k_dma_insts = []
            v_dma_insts = []

            def emit_qchunk(c):
                l0 = c * QC
                qt = qtp.tile([P, 4, QC], FP16, name="qt", tag="qt")
                gates = [] if c < UNGATED_QT else [k_dma_insts[-1], v_dma_insts[-1]]
                for g in range(4):
                    t_inst = nc.sync.dma_start(
                        qt[:, g, :],
                        q_d[l0 : l0 + QC, g * P : (g + 1) * P],
                        transpose=True,
                    )
                    if gates:
                        _ds = InstructionNameOrderedSet()
                        for g_ in gates:
                            _ds.add(g_.ins.name)
                        t_inst.ins.add_nosync_dependencies_from(_ds)
                qe = qep.tile([P, 4, QC], FP16, name=f"qm{c}", tag="qm")
                qr = qpp.tile([P, 4, QC], FP16, name=f"qr{c}", tag="qr")
                for half in range(2):
                    hsl = slice(half * QC // 2, (half + 1) * QC // 2)
                    nc.scalar.activation(qe[:, :, hsl], qt[:, :, hsl], AF.Exp)
                    nc.gpsimd.tensor_scalar_min(qe[:, :, hsl], qe[:, :, hsl], 1.0)
                    nc.vector.tensor_scalar_max(qr[:, :, hsl], qt[:, :, hsl], 0.0)
                qps.append((qe, qr))
                if debug and c == 0:
                    nc.sync.dma_start(qm_dbg[:, :], qe.rearrange("p g l -> p (g l)"))
                    nc.sync.dma_start(qr_dbg[:, :], qr.rearrange("p g l -> p (g l)"))
                    nc.sync.dma_start(qt_dbg[:, :], qt.rearrange("p g l -> p (g l)"))

            with tc.tile_pool(name="kvpsum", bufs=1, space="PSUM") as kvpsum:
                # ---- Phase 1: KV + Ksum accumulation ----
                kv_ps = [
                    kvpsum.tile([P, 129], FP32, name=f"kv{j}", tag=f"kv{j}")
                    for j in range(4)
                ]
                segs = [(c * KC, KC) for c in range(n_kc)]
                for si, (r0, rows) in enumerate(segs):
                    subs = rows // P
                    ktile = kdma.tile([P, subs, HD], FP16, name="ktile", tag="ktile")
                    k_inst = nc.sync.dma_start(
                        ktile,
                        k_d[r0 : r0 + rows, :].rearrange("(f p) d -> p f d", p=P),
                    )
                    k_dma_insts.append(k_inst)
                    vtile = vdma.tile([P, subs, HD], FP16, name="vtile", tag="vtile")
                    v_inst = nc.scalar.dma_start(
                        vtile,
                        v_d[r0 : r0 + rows, :].rearrange("(f p) d -> p f d", p=P),
                    )
                    v_dma_insts.append(v_inst)
                    # K' = elu(K)+1 = relu(K) + min(exp(K), 1)
                    ke = kep.tile([P, subs, HD], FP16, name="ke", tag="ke")
                    nc.scalar.activation(ke, ktile, AF.Exp)
                    nc.vector.tensor_scalar_min(ke, ke, 1.0)
                    kp = kpp.tile([P, subs, HD], FP16, name="kp", tag="kp")
                    nc.vector.scalar_tensor_tensor(
                        kp, in0=ktile, scalar=0.0, in1=ke, op0=OP.max, op1=OP.add
                    )
                    if debug and si == 0:
                        nc.sync.dma_start(kp_dbg[:, 0:subs, :], kp)
                        nc.sync.dma_start(ke_dbg[:, 0:subs, :], ke)
                    first = si == 0
                    last = si == len(segs) - 1
                    for sub in range(subs):
                        for j in range(4):
                            sl = slice(j * P, (j + 1) * P)
                            # NOTE: exactly ONE start=True per psum tile —
                            # start clears has_written for the whole bank row,
                            # so a second start would drop prior accumulation.
                            nc.tensor.matmul(
                                kv_ps[j][:, 0:128],
                                lhsT=kp[:, sub, sl],
                                rhs=vtile[:, sub, sl],
                                start=(first and sub == 0),
                                stop=(last and sub == subs - 1),
                            )
                            nc.tensor.matmul(
                                kv_ps[j][:, 128:129],
                                lhsT=kp[:, sub, sl],
                                rhs=ones,
                                start=False,
                                stop=(last and sub == subs - 1),
                            )
                # ---- Phase 1.5: block-diag [KV] fp16 weights + Ksum cols ----
                w2 = [
                    w2p.tile([P, 128], FP16, name=f"w2_{j}", tag=f"w2_{j}")
                    for j in range(4)
                ]
                ksc = [
                    w2p.tile([P, 2], FP16, name=f"ksc_{j}", tag=f"ksc_{j}")
                    for j in range(4)
                ]
                for j in range(4):
                    nc.vector.memset(w2[j], 0.0)
                    nc.vector.tensor_copy(w2[j][0:64, 0:64], kv_ps[j][0:64, 0:64])
                    nc.vector.tensor_copy(
                        w2[j][64:128, 64:128], kv_ps[j][64:128, 64:128]
                    )
                    nc.vector.memset(ksc[j], 0.0)
                    nc.vector.tensor_copy(ksc[j][0:64, 0:1], kv_ps[j][0:64, 128:129])
                    nc.vector.tensor_copy(
                        ksc[j][64:128, 1:2], kv_ps[j][64:128, 128:129]
                    )

            # ---- Q side: transpose + feature map for all chunks ----
            for qc in range(n_qc):
                emit_qchunk(qc)

            if debug:
                for j in range(4):
                    nc.sync.dma_start(w2_dbg[:, j * 128 : (j + 1) * 128], w2[j])
                    nc.sync.dma_start(ksc_dbg[:, j * 2 : (j + 1) * 2], ksc[j])

            # ---- Phase 2 (per qchunk): denominators -> recip -> out ----
            with (
                tc.tile_pool(name="dpsum", bufs=2, space="PSUM") as dpsum,
                tc.tile_pool(name="p2psum", bufs=3, space="PSUM") as p2psum,
            ):
                n_hc = n_qc
                lc_per_hc = QC // P  # 8
                for hc in range(n_hc):
                    qm, qr = qps[hc]
                    dps = dpsum.tile([P, 8 * lc_per_hc], FP32, name="dps", tag="dps")
                    for lc in range(lc_per_hc):
                        lsl = slice(lc * P, (lc + 1) * P)
                        for j in range(4):
                            for li, lh in enumerate((qm, qr)):
                                nc.tensor.matmul(
                                    dps[:, lc * 8 + 2 * j : lc * 8 + 2 * j + 2],
                                    lhsT=lh[:, j, lsl],
                                    rhs=ksc[j],
                                    start=(li == 0),
                                    stop=(li == 1),
                                )
                    zr = zrp.tile([P, 8 * lc_per_hc], FP32, name="zr", tag="zr")
                    nc.vector.reciprocal(zr, dps)
                    if debug and hc == 0:
                        nc.sync.dma_start(zr_dbg[:, :], zr)

                    otile = outp.tile([P, lc_per_hc, H, D], FP16, name="otile", tag="otile")
                    for lc2 in range(lc_per_hc // 2):
                        p2 = p2psum.tile([P, 2, 512], FP32, name="p2", tag="p2")
                        for half in range(2):
                            lc = 2 * lc2 + half
                            lsl = slice(lc * P, (lc + 1) * P)
                            for j in range(4):
                                for li, lh in enumerate((qm, qr)):
                                    nc.tensor.matmul(
                                        p2[:, half, j * 128 : (j + 1) * 128],
                                        lhsT=lh[:, j, lsl],
                                        rhs=w2[j],
                                        start=(li == 0),
                                        stop=(li == 1),
                                    )
                        zb = (
                            zr[:, lc2 * 16 : (lc2 + 1) * 16]
                            .rearrange("p (two e) -> p two e", two=2)
                            .unsqueeze(3)
                            .broadcast_to([P, 2, 8, D])
                        )
                        nc.vector.tensor_tensor(
                            otile[:, 2 * lc2 : 2 * lc2 + 2, :, :],
                            p2.rearrange("p two (h d) -> p two h d", h=H),
                            zb,
                            op=OP.mult,
                        )
                    r0 = hc * QC
                    nc.sync.dma_start(
                        o_d[r0 : r0 + QC, :].rearrange("(f p) d -> p f d", p=P),
                        otile.rearrange("p f h d -> p f (h d)"),
                    )
    nc.compile()
    return nc


@functools.lru_cache(maxsize=None)
def _cached_nc(L_, S_):
    return build_kernel(L_, S_)


def _par_cast(src, dst, n_threads=16):
    """dst[:] = src, chunked across threads (numpy astype releases the GIL)."""
    import concurrent.futures as cf

    n = src.shape[0]
    step = (n + n_threads - 1) // n_threads

    def work(i):
        dst[i : i + step] = src[i : i + step]

    with cf.ThreadPoolExecutor(n_threads) as ex:
        list(ex.map(work, range(0, n, step)))
    return dst


class _Runner:
    """Persistent jitted SPMD runner (see git history for rationale):
    no per-call retrace, no concatenate (reshape views), no zero-buffer
    upload (cycles the previous output as the donated buffer), single
    asarray+reshape on the way out."""

    def __init__(self, nc, n_cores):
        import jax
        from jax.sharding import Mesh, NamedSharding, PartitionSpec
        from jax.experimental.shard_map import shard_map
        from concourse.bass2jax import (
            _bass_exec_p,
            install_neuronx_cc_hook,
            partition_id_tensor,
        )

        install_neuronx_cc_hook()
        self.nc = nc
        self.n_cores = n_cores

        partition_name = (
            nc.partition_id_tensor.name if nc.partition_id_tensor else None
        )
        in_names, out_names, out_avals = [], [], []
        for alloc in nc.m.functions[0].allocations:
            if not isinstance(alloc, mybir.MemoryLocationSet):
                continue
            name = alloc.memorylocations[0].name
            if alloc.kind == "ExternalInput":
                if name != partition_name:
                    in_names.append(name)
            elif alloc.kind == "ExternalOutput":
                out_names.append(name)
                out_avals.append(
                    jax.core.ShapedArray(
                        tuple(alloc.tensor_shape), mybir.dt.np(alloc.dtype)
                    )
                )
        self.in_names = list(in_names)
        self.out_names = list(out_names)
        self.out_avals = out_avals
        n_params = len(in_names)
        n_outs = len(out_names)
        in_names_full = in_names + out_names
        if partition_name is not None:
            in_names_full.append(partition_name)

        def _body(*args):
            operands = list(args)
            if partition_name is not None:
                operands.append(partition_id_tensor())
            return tuple(
                _bass_exec_p.bind(
                    *operands,
                    out_avals=tuple(out_avals),
                    in_names=tuple(in_names_full),
                    out_names=tuple(out_names),
                    lowering_input_output_aliases=(),
                    sim_require_finite=True,
                    sim_require_nnan=True,
                    nc=nc,
                )
            )

        devices = jax.devices()[:n_cores]
        self.mesh = Mesh(np.asarray(devices), ("core",))
        self.shard = NamedSharding(self.mesh, PartitionSpec("core"))
        in_specs = (PartitionSpec("core"),) * (n_params + n_outs)
        out_specs = (PartitionSpec("core"),) * n_outs
        self.sharded = jax.jit(
            shard_map(
                _body,
                mesh=self.mesh,
                in_specs=in_specs,
                out_specs=out_specs,
                check_rep=False,
            ),
            donate_argnums=tuple(range(n_params, n_params + n_outs)),
            keep_unused=True,
        )
        import jax.numpy as jnp

        self._donate_bufs = [
            jax.jit(
                functools.partial(
                    jnp.zeros, (n_cores * a.shape[0], *a.shape[1:]), a.dtype
                ),
                out_shardings=self.shard,
            )()
            for a in out_avals
        ]

    def __call__(self, arrs: dict) -> list:
        ins = [arrs[nm] for nm in self.in_names]
        outs = self.sharded(*ins, *self._donate_bufs)
        self._donate_bufs = list(outs)
        return [np.asarray(o) for o in outs]


@functools.lru_cache(maxsize=None)
def _cached_runner(L_, S_):
    return _Runner(_cached_nc(L_, S_), N)


def kernel(queries: np.ndarray, keys: np.ndarray, values: np.ndarray) -> np.ndarray:
    n, l_, h, d = queries.shape
    s_ = keys.shape[1]
    runner = _cached_runner(l_, s_)
    hd = h * d
    arrs = {}
    for nm, full in (("queries", queries), ("keys", keys), ("values", values)):
        rows = full.shape[1]
        src = np.ascontiguousarray(full, np.float32).reshape(n * rows, hd)
        arrs[nm] = _par_cast(src, np.empty((n * rows, hd), np.float16))
    out16 = runner(arrs)[0]
    out32 = _par_cast(out16, np.empty(out16.shape, np.float32))
    return out32.reshape(n, l_, h, d)


if __name__ == "__main__":
    nc = build_kernel()
    print("build ok")
    from concourse.timeline_sim import TimelineSim

    print("sim ns:", TimelineSim(nc).simulate())


# revision 61
# speedup vs baseline: 1.1277x; 1.1277x over previous
"""Linear attention (elu+1 feature map) Bass/Tile kernel for Trainium2.

Full inputs: queries/keys/values [N=8, L/S=8192, H=8, D=64] fp32.
Sharding: data-parallel over N across the 8 NeuronCores (batch i -> core i).
Wire format: fp16 (inputs cast on host, output cast back), halving both
the host<->device transfer and the on-device HBM traffic.

Math per (n, h):
  Q' = elu(Q)+1, K' = elu(K)+1      with elu(x)+1 == relu(x) + min(exp(x), 1)
  KV[d, v] = sum_s K'[s, d] V[s, v]  (the /S, *S in the reference cancel)
  Ksum[d]  = sum_s K'[s, d]
  out[l, v] = (Q'[l, :] @ KV[:, v]) / (Q'[l, :] @ Ksum)
  (EPS=1e-6 is dropped: denominators are >= ~1e3 for any input since
   K', Q' > 0 and Ksum ~ S.)

Kernel structure per core (all DMAs issued from SP except V on ACT):
  Phase 1 (stream K/V, 1024-row chunks): contiguous fp16 DMAs; feature map
    K' = relu(K) + min(exp(K),1) as ACT Exp + DVE tensor_scalar_min (4x) +
    DVE scalar_tensor_tensor (max,add); per 128-row sub-block and head pair
    j, two accumulating matmuls into PSUM kv_ps[j] [128, 129]:
    lhsT=K'_pair x rhs=V_pair (block-diagonal KV) and x rhs=ones (Ksum col).
    NOTE: exactly one start=True per psum tile - start clears has_written
    for the whole bank row, so a second start drops prior accumulation.
  Phase 1.5: cast PSUM -> fp16 block-diag weights w2[j] [128,128] and
    Ksum columns ksc[j] [128,2].
  Phase Q (per 1024 rows, DMA-gated behind the K/V stream via nosync deps):
    DMA-TRANSPOSE Q to [128 f, l] fp16 (16x128 XBAR tiles, ~14ns/tile);
    feature map split as M = min(exp,1) (ACT + GpSimd ts_min) and
    R = relu (DVE ts_max 4x); the "+" of Q' = M + R is folded into phase 2
    as a second accumulating matmul per output (lhsT=M then lhsT=R).
  Phase 2 per 1024-row chunk: 64 tiny denominator matmuls (rhs=ksc) into
    one PSUM bank -> one wide DVE reciprocal -> zr [128, 64] f32; per 256
    rows one p2 PSUM tile [128, 2, 512] (8 matmuls, heads in column order)
    and ONE DVE tensor_tensor multiply against zr broadcast [128,2,8,64]
    -> fp16 out staging; one fp16 out DMA per 1024 rows.

TimelineSim per-core: ~128.8us (baseline kernel: 317.6us).
"""

import functools
import sys

sys.path.insert(0, "/opt/trn_rl_repo")

import numpy as np

import concourse.bass as bass
from concourse.bass import InstructionNameOrderedSet
import concourse.mybir as mybir
import concourse.tile as tile
from concourse import bacc

N, L, S, H, D = 8, 8192, 8192, 8, 64
P = 128
HD = H * D  # 512
FP32 = mybir.dt.float32
FP16 = mybir.dt.float16
AF = mybir.ActivationFunctionType
OP = mybir.AluOpType

UNGATED_QT = 0  # leading Q chunks allowed to interleave with K/V
KC = 1024  # K/V rows per chunk
QC = 1024  # Q rows per transpose chunk
OC = 1024  # out rows per DMA


def build_kernel(L_=L, S_=S, debug=False):
    nc = bacc.Bacc(trn_type="TRN2")
    q_d = nc.dram_tensor("queries", [L_, HD], FP16, kind="ExternalInput")
    k_d = nc.dram_tensor("keys", [S_, HD], FP16, kind="ExternalInput")
    v_d = nc.dram_tensor("values", [S_, HD], FP16, kind="ExternalInput")
    o_d = nc.dram_tensor("out", [L_, HD], FP16, kind="ExternalOutput")
    if debug:
        w2_dbg = nc.dram_tensor("w2_dbg", [P, 512], FP16, kind="ExternalOutput")
        ksc_dbg = nc.dram_tensor("ksc_dbg", [P, 8], FP16, kind="ExternalOutput")
        zr_dbg = nc.dram_tensor("zr_dbg", [P, 64], FP32, kind="ExternalOutput")
        qm_dbg = nc.dram_tensor("qm_dbg", [P, 4 * 1024], FP16, kind="ExternalOutput")
        qr_dbg = nc.dram_tensor("qr_dbg", [P, 4 * 1024], FP16, kind="ExternalOutput")
        qt_dbg = nc.dram_tensor("qt_dbg", [P, 4 * 1024], FP16, kind="ExternalOutput")
        kp_dbg = nc.dram_tensor("kp_dbg", [P, 8, HD], FP16, kind="ExternalOutput")
        ke_dbg = nc.dram_tensor("ke_dbg", [P, 8, HD], FP16, kind="ExternalOutput")

    n_kc = S_ // KC  # 16
    n_qc = L_ // QC  # 8
    lc_per_qc = QC // P  # 8 query sub-chunks of 128 per qchunk
    kc_subs = KC // P  # 4

    with tile.TileContext(nc) as tc:
        with (
            tc.tile_pool(name="consts", bufs=1) as consts,
            tc.tile_pool(name="kdma", bufs=4) as kdma,
            tc.tile_pool(name="vdma", bufs=3) as vdma,
            tc.tile_pool(name="ke", bufs=2) as kep,
            tc.tile_pool(name="kp", bufs=2) as kpp,
            tc.tile_pool(name="qt", bufs=3) as qtp,
            tc.tile_pool(name="qe", bufs=4) as qep,
            tc.tile_pool(name="qp", bufs=4) as qpp,
            tc.tile_pool(name="w2", bufs=1) as w2p,
            tc.tile_pool(name="zr", bufs=3) as zrp,
            tc.tile_pool(name="otile", bufs=3) as outp,
        ):
            ones = consts.tile([P, 1], FP16)
            nc.vector.memset(ones, 1.0)

            qps = []
            k_dma_insts = []
            v_dma_insts = []

            def emit_qchunk(c):
                l0 = c * QC
                qt = qtp.tile([P, 4, QC], FP16, name="qt", tag="qt")
                gates = [] if c < UNGATED_QT else [k_dma_insts[-1], v_dma_insts[-1]]
                for g in range(4):
                    t_inst = nc.sync.dma_start(
                        qt[:, g, :],
                        q_d[l0 : l0 + QC, g * P : (g + 1) * P],
                        transpose=True,
                    )
                    if gates:
                        _ds = InstructionNameOrderedSet()
                        for g_ in gates:
                            _ds.add(g_.ins.name)
                        t_inst.ins.add_nosync_dependencies_from(_ds)
                qe = qep.tile([P, 4, QC], FP16, name=f"qm{c}", tag="qm")
                qr = qpp.tile([P, 4, QC], FP16, name=f"qr{c}", tag="qr")
                for half in range(2):
                    hsl = slice(half * QC // 2, (half + 1) * QC // 2)
                    nc.scalar.activation(qe[:, :, hsl], qt[:, :, hsl], AF.Exp)
                    nc.gpsimd.tensor_scalar_min(qe[:, :, hsl], qe[:, :, hsl], 1.0)
                    nc.vector.tensor_scalar_max(qr[:, :, hsl], qt[:, :, hsl], 0.0)
                qps.append((qe, qr))
                if debug and c == 0:
                    nc.sync.dma_start(qm_dbg[:, :], qe.rearrange("p g l -> p (g l)"))
                    nc.sync.dma_start(qr_dbg[:, :], qr.rearrange("p g l -> p (g l)"))
                    nc.sync.dma_start(qt_dbg[:, :], qt.rearrange("p g l -> p (g l)"))

            with tc.tile_pool(name="kvpsum", bufs=1, space="PSUM") as kvpsum:
                # ---- Phase 1: KV + Ksum accumulation ----
                kv_ps = [
                    kvpsum.tile([P, 129], FP32, name=f"kv{j}", tag=f"kv{j}")
                    for j in range(4)
                ]
                segs = [(c * KC, KC) for c in range(n_kc)]
                for si, (r0, rows) in enumerate(segs):
                    subs = rows // P
                    ktile = kdma.tile([P, subs, HD], FP16, name="ktile", tag="ktile")
                    k_inst = nc.sync.dma_start(
                        ktile,
                        k_d[r0 : r0 + rows, :].rearrange("(f p) d -> p f d", p=P),
                    )
                    k_dma_insts.append(k_inst)
                    vtile = vdma.tile([P, subs, HD], FP16, name="vtile", tag="vtile")
                    v_inst = nc.scalar.dma_start(
                        vtile,
                        v_d[r0 : r0 + rows, :].rearrange("(f p) d -> p f d", p=P),
                    )
                    v_dma_insts.append(v_inst)
                    # K' = elu(K)+1 = relu(K) + min(exp(K), 1)
                    ke = kep.tile([P, subs, HD], FP16, name="ke", tag="ke")
                    nc.scalar.activation(ke, ktile, AF.Exp)
                    nc.vector.tensor_scalar_min(ke, ke, 1.0)
                    kp = kpp.tile([P, subs, HD], FP16, name="kp", tag="kp")
                    nc.vector.scalar_tensor_tensor(
                        kp, in0=ktile, scalar=0.0, in1=ke, op0=OP.max, op1=OP.add
                    )
                    if debug and si == 0:
                        nc.sync.dma_start(kp_dbg[:, 0:subs, :], kp)
                        nc.sync.dma_start(ke_dbg[:, 0:subs, :], ke)
                    first = si == 0
                    last = si == len(segs) - 1
                    for sub in range(subs):
                        for j in range(4):
                            sl = slice(j * P, (j + 1) * P)
                            # NOTE: exactly ONE start=True per psum tile —
                            # start clears has_written for the whole bank row,
                            # so a second start would drop prior accumulation.
                            nc.tensor.matmul(
                                kv_ps[j][:, 0:128],
                                lhsT=kp[:, sub, sl],
                                rhs=vtile[:, sub, sl],
                                start=(first and sub == 0),
                                stop=(last and sub == subs - 1),
                            )
                            nc.tensor.matmul(
                                kv_ps[j][:, 128:129],
                                lhsT=kp[:, sub, sl],
                                rhs=ones,
                                start=False,
                                stop=(last and sub == subs - 1),
                            )
                # ---- Phase 1.5: block-diag [KV] fp16 weights + Ksum cols ----
                w2 = [
                    w2p.tile([P, 128], FP16, name=f"w2_{j}", tag=f"w2_{j}")
                    for j in range(4)
                ]
                ksc = [
                    w2p.tile([P, 2], FP16, name=f"ksc_{j}", tag=f"ksc_{j}")
                    for j in range(4)
                ]
                for j in range(4):
                    nc.vector.memset(w2[j], 0.0)
                    nc.vector.tensor_copy(w2[j][0:64, 0:64], kv_ps[j][0:64, 0:64])
                    nc.vector.tensor_copy(
                        w2[j][64:128, 64:128], kv_ps[j][64:128, 64:128]
                    )
                    nc.vector.memset(ksc[j], 0.0)
                    nc.vector.tensor_copy(ksc[j][0:64, 0:1], kv_ps[j][0:64, 128:129])
                    nc.vector.tensor_copy(
                        ksc[j][64:128, 1:2], kv_ps[j][64:128, 128:129]
                    )

            # ---- Q side: transpose + feature map for all chunks ----
            for qc in range(n_qc):
                emit_qchunk(qc)

            if debug:
                for j in range(4):
                    nc.sync.dma_start(w2_dbg[:, j * 128 : (j + 1) * 128], w2[j])
                    nc.sync.dma_start(ksc_dbg[:, j * 2 : (j + 1) * 2], ksc[j])

            # ---- Phase 2 (per qchunk): denominators -> recip -> out ----
            with (
                tc.tile_pool(name="dpsum", bufs=2, space="PSUM") as dpsum,
                tc.tile_pool(name="p2psum", bufs=3, space="PSUM") as p2psum,
            ):
                n_hc = n_qc
                lc_per_hc = QC // P  # 8
                for hc in range(n_hc):
                    qm, qr = qps[hc]
                    dps = dpsum.tile([P, 8 * lc_per_hc], FP32, name="dps", tag="dps")
                    for lc in range(lc_per_hc):
                        lsl = slice(lc * P, (lc + 1) * P)
                        for j in range(4):
                            for li, lh in enumerate((qm, qr)):
                                nc.tensor.matmul(
                                    dps[:, lc * 8 + 2 * j : lc * 8 + 2 * j + 2],
                                    lhsT=lh[:, j, lsl],
                                    rhs=ksc[j],
                                    start=(li == 0),
                                    stop=(li == 1),
                                )
                    zr = zrp.tile([P, 8 * lc_per_hc], FP32, name="zr", tag="zr")
                    nc.vector.reciprocal(zr, dps)
                    if debug and hc == 0:
                        nc.sync.dma_start(zr_dbg[:, :], zr)

                    otile = outp.tile([P, lc_per_hc, H, D], FP16, name="otile", tag="otile")
                    for lc2 in range(lc_per_hc // 2):
                        p2 = p2psum.tile([P, 2, 512], FP32, name="p2", tag="p2")
                        for half in range(2):
                            lc = 2 * lc2 + half
                            lsl = slice(lc * P, (lc + 1) * P)
                            for j in range(4):
                                for li, lh in enumerate((qm, qr)):
                                    nc.tensor.matmul(
                                        p2[:, half, j * 128 : (j + 1) * 128],
                                        lhsT=lh[:, j, lsl],
                                        rhs=w2[j],
                                        start=(li == 0),
                                        stop=(li == 1),
                                    )
                        zb = (
                            zr[:, lc2 * 16 : (lc2 + 1) * 16]
                            .rearrange("p (two e) -> p two e", two=2)
                            .unsqueeze(3)
                            .broadcast_to([P, 2, 8, D])
                        )
                        nc.vector.tensor_tensor(
                            otile[:, 2 * lc2 : 2 * lc2 + 2, :, :],
                            p2.rearrange("p two (h d) -> p two h d", h=H),
                            zb,
                            op=OP.mult,
                        )
                    r0 = hc * QC
                    nc.sync.dma_start(
                        o_d[r0 : r0 + QC, :].rearrange("(f p) d -> p f d", p=P),
                        otile.rearrange("p f h d -> p f (h d)"),
                    )
    nc.compile()
    return nc


@functools.lru_cache(maxsize=None)
def _cached_nc(L_, S_):
    return build_kernel(L_, S_)


def _par_cast(src, dst, n_threads=16):
    """dst[:] = src, chunked across threads (numpy astype releases the GIL)."""
    import concurrent.futures as cf

    n = src.shape[0]
    step = (n + n_threads - 1) // n_threads

    def work(i):
        dst[i : i + step] = src[i : i + step]

    with cf.ThreadPoolExecutor(n_threads) as ex:
        list(ex.map(work, range(0, n, step)))
    return dst


class _Runner:
    """Persistent jitted SPMD runner (see git history for rationale):
    no per-call retrace, no concatenate (reshape views), no zero-buffer
    upload (cycles the previous output as the donated buffer), single
    asarray+reshape on the way out."""

    def __init__(self, nc, n_cores):
        import jax
        from jax.sharding import Mesh, NamedSharding, PartitionSpec
        from jax.experimental.shard_map import shard_map
        from concourse.bass2jax import (
            _bass_exec_p,
            install_neuronx_cc_hook,
            partition_id_tensor,
        )

        install_neuronx_cc_hook()
        self.nc = nc
        self.n_cores = n_cores

        partition_name = (
            nc.partition_id_tensor.name if nc.partition_id_tensor else None
        )
        in_names, out_names, out_avals = [], [], []
        for alloc in nc.m.functions[0].allocations:
            if not isinstance(alloc, mybir.MemoryLocationSet):
                continue
            name = alloc.memorylocations[0].name
            if alloc.kind == "ExternalInput":
                if name != partition_name:
                    in_names.append(name)
            elif alloc.kind == "ExternalOutput":
                out_names.append(name)
                out_avals.append(
                    jax.core.ShapedArray(
                        tuple(alloc.tensor_shape), mybir.dt.np(alloc.dtype)
                    )
                )
        self.in_names = list(in_names)
        self.out_names = list(out_names)
        self.out_avals = out_avals
        n_params = len(in_names)
        n_outs = len(out_names)
        in_names_full = in_names + out_names
        if partition_name is not None:
            in_names_full.append(partition_name)

        def _body(*args):
            operands = list(args)
            if partition_name is not None:
                operands.append(partition_id_tensor())
            return tuple(
                _bass_exec_p.bind(
                    *operands,
                    out_avals=tuple(out_avals),
                    in_names=tuple(in_names_full),
                    out_names=tuple(out_names),
                    lowering_input_output_aliases=(),
                    sim_require_finite=True,
                    sim_require_nnan=True,
                    nc=nc,
                )
            )

        devices = jax.devices()[:n_cores]
        self.mesh = Mesh(np.asarray(devices), ("core",))
        self.shard = NamedSharding(self.mesh, PartitionSpec("core"))
        in_specs = (PartitionSpec("core"),) * (n_params + n_outs)
        out_specs = (PartitionSpec("core"),) * n_outs
        self.sharded = jax.jit(
            shard_map(
                _body,
                mesh=self.mesh,
                in_specs=in_specs,
                out_specs=out_specs,
                check_rep=False,
            ),
            donate_argnums=tuple(range(n_params, n_params + n_outs)),
            keep_unused=True,
        )
        import jax.numpy as jnp

        self._donate_bufs = [
            jax.jit(
                functools.partial(
                    jnp.zeros, (n_cores * a.shape[0], *a.shape[1:]), a.dtype
                ),
                out_shardings=self.shard,
            )()
            for a in out_avals
        ]

    def __call__(self, arrs: dict) -> list:
        ins = [arrs[nm] for nm in self.in_names]
        outs = self.sharded(*ins, *self._donate_bufs)
        self._donate_bufs = list(outs)
        return [np.asarray(o) for o in outs]


@functools.lru_cache(maxsize=None)
def _cached_runner(L_, S_):
    return _Runner(_cached_nc(L_, S_), N)


def kernel(queries: np.ndarray, keys: np.ndarray, values: np.ndarray) -> np.ndarray:
    n, l_, h, d = queries.shape
    s_ = keys.shape[1]
    runner = _cached_runner(l_, s_)
    hd = h * d
    arrs = {}
    for nm, full in (("queries", queries), ("keys", keys), ("values", values)):
        rows = full.shape[1]
        src = np.ascontiguousarray(full, np.float32).reshape(n * rows, hd)
        arrs[nm] = _par_cast(src, np.empty((n * rows, hd), np.float16))
    out16 = runner(arrs)[0]
    out32 = _par_cast(out16, np.empty(out16.shape, np.float32))
    return out32.reshape(n, l_, h, d)


if __name__ == "__main__":
    nc = build_kernel()
    print("build ok")
    from concourse.timeline_sim import TimelineSim

    print("sim ns:", TimelineSim(nc).simulate())
